# revision 25
# baseline (speedup 1.0000x reference)
"""CRF forward (partition function) kernel for Trainium2, 8 NeuronCores.

Meet-in-the-middle formulation (exp space), data-parallel over batch:
  forward   F_{i+1} = ef_i * (W @ F_i),            i = 0..M-1   (alpha side)
  backward  G_t = W^T @ (ef_t * G_{t+1}) + 1[length==t] * exp(trans[END]),
run from both ends to the midpoint M = S/2 (lengths >= S/2, so the forward
half is mask-free); host combines out[b] = log(F_M . G_M) + accumulators.

W[next,prev] = exp(trans[next,prev]); ef is exp(feat - max_tag feat) (host
prescale, bookkept via cumsum); every NK steps the device renormalizes each
batch column by r ~ 1/colsum (computed on-device, applied to a later ef
slice, exact r values dumped for host compensation).

The backward injection rides inside the one matmul per step: the state is
augmented with 3 extra rows -- row 64 a self-perpetuating constant 1, rows
65/66 per-tag-group injection markers delivered via the ef stream (marker
row at time t = 1[length==t]); the stationary has columns that (a) copy the
constant row forward and (b) add exp(trans[END])[prev] * marker to each
group's state rows.  No extra instructions, no PSUM read-modify-write.

Layout per chain: 2 tag-groups of 32 tags stacked on partitions, 64 batch
elems on the free dim; one chain per direction (forward 64 partitions,
backward 67).  The serial critical path per chain step is the PE->DVE
semaphore round trip (~500ns); the two chains interleave on the engines.
"""

import os
import sys

import numpy as np
import ml_dtypes

if "/opt/trn_rl_repo" not in sys.path:
    sys.path.insert(0, "/opt/trn_rl_repo")

import concourse.bass as bass
import concourse.tile as tile
from concourse import bacc, mybir
from concourse.bass_utils import run_bass_kernel_spmd

BF = ml_dtypes.bfloat16
S, B, T = 1024, 1024, 32
START, END = T - 2, T - 1
NCORES = 8
BC = B // NCORES            # batch per core (128)
NK, EV0, LAG = 16, 4, 3     # renorm cadence / first event / apply lag
CHUNK = 128                 # steps per DMA chunk
P, NGRP, FD = 64, 2, 64     # partitions (tags), tag groups, batch free dim
PB = P + 3                  # backward partitions (+const row, +2 markers)

dt = mybir.dt


def build_program(s_len=S):
    """One SPMD program for all cores: forward + backward half-chains."""
    m = s_len // 2
    chunk = min(CHUNK, m)
    n_ev = (m - EV0 - 1) // NK + 1 if m > EV0 else 0

    nc = bacc.Bacc("TRN2", target_bir_lowering=False, num_devices=NCORES)

    efF_d = nc.dram_tensor("efF", [P, m * FD], dt.bfloat16, kind="ExternalInput")
    efB_d = nc.dram_tensor("efB", [PB, m * FD], dt.bfloat16, kind="ExternalInput")
    y0_d = nc.dram_tensor("y0", [PB, FD], dt.bfloat16, kind="ExternalInput")
    qiF_d = nc.dram_tensor("qinitF", [P, FD], dt.bfloat16, kind="ExternalInput")
    wF_d = nc.dram_tensor("wblkF", [P, P], dt.bfloat16, kind="ExternalInput")
    wB_d = nc.dram_tensor("wblkB", [PB, PB], dt.bfloat16, kind="ExternalInput")
    obF_d = nc.dram_tensor("onesblkF", [P, NGRP], dt.bfloat16, kind="ExternalInput")
    obB_d = nc.dram_tensor("onesblkB", [PB, NGRP], dt.bfloat16, kind="ExternalInput")
    oc_d = nc.dram_tensor("onesbc", [NGRP, P], dt.bfloat16, kind="ExternalInput")

    qF_o = nc.dram_tensor("qF", [P, FD], dt.bfloat16, kind="ExternalOutput")
    qB_o = nc.dram_tensor("qB", [P, FD], dt.bfloat16, kind="ExternalOutput")
    rdF_o = nc.dram_tensor("rdF", [NGRP, max(1, n_ev) * FD], dt.bfloat16,
                           kind="ExternalOutput")
    rdB_o = nc.dram_tensor("rdB", [NGRP, max(1, n_ev) * FD], dt.bfloat16,
                           kind="ExternalOutput")

    with tile.TileContext(nc) as tc:
        with (
            tc.tile_pool(name="singles", bufs=1) as singles,
            tc.tile_pool(name="efpool", bufs=2) as efpool,
            tc.tile_pool(name="small", bufs=2) as small,
            tc.tile_pool(name="ypool", bufs=4) as ypool,
            tc.tile_pool(name="fpool", bufs=4) as fpool,
            tc.tile_pool(name="psF", bufs=3, space="PSUM") as psf_pool,
            tc.tile_pool(name="psB", bufs=3, space="PSUM") as psb_pool,
            tc.tile_pool(name="psE", bufs=1, space="PSUM") as pse_pool,
        ):
            wF_t = singles.tile([P, P], dt.bfloat16, tag="wF", name="wF_t")
            wB_t = singles.tile([PB, PB], dt.bfloat16, tag="wB", name="wB_t")
            obF_t = singles.tile([P, NGRP], dt.bfloat16, tag="obF", name="obF_t")
            obB_t = singles.tile([PB, NGRP], dt.bfloat16, tag="obB", name="obB_t")
            oc_t = singles.tile([NGRP, P], dt.bfloat16, tag="oc", name="oc_t")
            for tl, dr in ((wF_t, wF_d), (wB_t, wB_d), (obF_t, obF_d),
                           (obB_t, obB_d), (oc_t, oc_d)):
                nc.sync.dma_start(out=tl, in_=dr.ap())

            rbF = singles.tile([NGRP, max(1, n_ev) * FD], dt.bfloat16,
                               tag="rbF", name="rbF")
            rbB = singles.tile([NGRP, max(1, n_ev) * FD], dt.bfloat16,
                               tag="rbB", name="rbB")
            f_cur = fpool.tile([P, FD], dt.bfloat16, tag="f", name="f_0")
            nc.sync.dma_start(out=f_cur, in_=qiF_d.ap())

            y_cur = ypool.tile([PB, FD], dt.bfloat16, tag="y", name="y_0")
            nc.sync.dma_start(out=y_cur, in_=y0_d.ap())

            chF = [None, None]
            chB = [None, None]
            pendF, pendB = {}, {}

            def event(i, cur, ob_t, rbuf, pend, is_b):
                e = (i - EV0) // NK
                psc = pse_pool.tile([NGRP, FD], dt.float32, tag="psC",
                                    name=f"psC{int(is_b)}_{i}")
                nc.tensor.matmul(psc, ob_t, cur, start=True, stop=True)
                rf = small.tile([NGRP, FD], dt.float32, tag="rf",
                                name=f"rf{int(is_b)}_{i}")
                nc.vector.reciprocal_approx_fast(out=rf, in_=psc)
                rsb = rbuf[:, e * FD:(e + 1) * FD]
                nc.vector.tensor_copy(rsb, rf)
                psr = pse_pool.tile([P, FD], dt.float32, tag="psR",
                                    name=f"psR{int(is_b)}_{i}")
                nc.tensor.matmul(psr, oc_t, rsb, start=True, stop=True)
                if i + LAG < (m - 1 if is_b else m):
                    pend[i + LAG] = psr

            n_chunks = (m + chunk - 1) // chunk
            for ch in range(n_chunks):
                tF = efpool.tile([P, chunk * FD], dt.bfloat16, tag="efF",
                                 name=f"efF_{ch}")
                nc.sync.dma_start(
                    out=tF, in_=efF_d.ap()[:, ch * chunk * FD:
                                           (ch + 1) * chunk * FD])
                chF[ch % 2] = tF
                tB = efpool.tile([PB, chunk * FD], dt.bfloat16, tag="efB",
                                 name=f"efB_{ch}")
                nc.sync.dma_start(
                    out=tB, in_=efB_d.ap()[:, ch * chunk * FD:
                                           (ch + 1) * chunk * FD])
                chB[ch % 2] = tB

                for i in range(ch * chunk, min((ch + 1) * chunk, m)):
                    csl = slice((i % chunk) * FD, (i % chunk) * FD + FD)
                    # ---------------- forward chain, step i -----------------
                    curF = f_cur
                    if i >= EV0 and (i - EV0) % NK == 0:
                        event(i, curF, obF_t, rbF, pendF, is_b=False)
                    eslF = chF[(i // chunk) % 2][:, csl]
                    if i in pendF:
                        psr = pendF.pop(i)
                        efx = small.tile([P, FD], dt.bfloat16, tag="efxF",
                                         name=f"efxF_{i}")
                        nc.vector.tensor_mul(efx, psr, eslF)
                        eslF = efx
                    psf = psf_pool.tile([P, FD], dt.float32, tag="psf",
                                        name=f"psf_{i}")
                    nc.tensor.matmul(psf, wF_t, curF, start=True, stop=True)
                    nxtF = fpool.tile([P, FD], dt.bfloat16, tag="f",
                                      name=f"f_{i + 1}")
                    nc.vector.tensor_mul(nxtF, psf, eslF)
                    f_cur = nxtF

                    # ---------------- backward chain, step i ----------------
                    if i >= EV0 and (i - EV0) % NK == 0:
                        event(i, y_cur, obB_t, rbB, pendB, is_b=True)
                    psb = psb_pool.tile([PB, FD], dt.float32, tag="psb",
                                        name=f"psb_{i}")
                    nc.tensor.matmul(psb, wB_t, y_cur, start=True, stop=True)
                    if i < m - 1:
                        eslB = chB[(i // chunk) % 2][:, csl]
                        if i in pendB:
                            psr = pendB.pop(i)
                            efx = small.tile([PB, FD], dt.bfloat16, tag="efxB",
                                             name=f"efxB_{i}")
                            nc.vector.tensor_mul(efx[0:P, :], psr, eslB[0:P, :])
                            nc.vector.tensor_copy(efx[P:PB, :], eslB[P:PB, :])
                            eslB = efx
                        y_nxt = ypool.tile([PB, FD], dt.bfloat16, tag="y",
                                           name=f"y_{i + 1}")
                        nc.vector.tensor_mul(y_nxt, psb, eslB)
                        y_cur = y_nxt
                    else:
                        qB_t = singles.tile([P, FD], dt.bfloat16, tag="qBf",
                                            name="qB_t")
                        nc.vector.tensor_copy(qB_t, psb[0:P, :])

            nc.sync.dma_start(out=qF_o.ap(), in_=f_cur)
            nc.sync.dma_start(out=qB_o.ap(), in_=qB_t)
            nc.sync.dma_start(out=rdF_o.ap(), in_=rbF)
            nc.sync.dma_start(out=rdB_o.ap(), in_=rbB)

    nc.finalize()
    return nc


def _host_prep(feats, transition, lengths):
    """Per-core in_maps plus reconstruction metadata."""
    s_len, b_tot = feats.shape[0], feats.shape[1]
    n_cores = b_tot // BC
    m = s_len // 2
    c_pre = feats.max(axis=2)                                # (S, B)
    Ccum = np.vstack([np.zeros((1, b_tot), np.float64),
                      np.cumsum(c_pre.astype(np.float64), 0)])  # (S+1, B)
    ef = np.exp(feats - c_pre[:, :, None]).astype(BF)        # (S, B, T)

    W = np.exp(transition.astype(np.float64))                # [next, prev]
    lhsF = W.T.astype(BF).astype(np.float32)                 # [prev, next]
    lhsB = W.astype(BF).astype(np.float32)                   # [next, prev]
    eT = np.exp(transition[END].astype(np.float64))          # (T,)
    eTb = eT.astype(BF).astype(np.float32)

    wF = np.zeros((P, P), np.float32)
    wB = np.zeros((PB, PB), np.float32)
    for gi in range(NGRP):
        s32 = slice(gi * 32, (gi + 1) * 32)
        wF[s32, s32] = lhsF
        wB[s32, s32] = lhsB
        wB[P + 1 + gi, s32] = eTb                # marker row g -> inject eT
    wB[P, P:PB] = 1.0                            # const row perpetuates
    obF = np.zeros((P, NGRP), np.float32)
    obB = np.zeros((PB, NGRP), np.float32)
    onesbc = np.zeros((NGRP, P), np.float32)
    for gi in range(NGRP):
        obF[gi * 32:(gi + 1) * 32, gi] = 1.0
        obB[gi * 32:(gi + 1) * 32, gi] = 1.0
        onesbc[gi, gi * 32:(gi + 1) * 32] = 1.0
    obB[P, :] = 1.0                              # colsum += 1 (zero-col guard)

    qinitF = np.zeros((P, FD), np.float32)
    qinitF[START, :] = 1.0
    qinitF[32 + START, :] = 1.0

    in_maps = []
    for core in range(n_cores):
        sl = slice(core * BC, (core + 1) * BC)
        A = ef[:, sl, :]                                     # (S, 128, T)
        # brick: [g*32+tag, t, bi] = A[t, g*FD+bi, tag]
        E = (A.reshape(s_len, NGRP, FD, T).transpose(1, 3, 0, 2)
             .reshape(P, s_len, FD)).astype(np.float32)
        EF = np.ascontiguousarray(E[:, :m, :]).reshape(P, m * FD)
        Lc = lengths[sl].astype(int)                         # (128,)
        mark = np.zeros((NGRP, s_len + 1, FD), np.float32)   # [g, t, bi]
        for gi in range(NGRP):
            for bi in range(FD):
                mark[gi, Lc[gi * FD + bi], bi] = 1.0
        # backward stream col i <- t = s_len-2-i, rows: ef, 1, markers at t
        EB = np.zeros((PB, m, FD), np.float32)
        ts = s_len - 2 - np.arange(m)                        # (m,)
        EB[:P] = E[:, ts, :]
        EB[P] = 1.0
        EB[P + 1] = mark[0, ts, :]
        EB[P + 2] = mark[1, ts, :]
        EB = np.ascontiguousarray(EB).reshape(PB, m * FD)
        # y_0: rows = qinitB * ef_{S-1}, const 1, markers at t = S-1
        y0 = np.zeros((PB, FD), np.float32)
        for gi in range(NGRP):
            live = (Lc[gi * FD:(gi + 1) * FD] == s_len).astype(np.float32)
            y0[gi * 32:(gi + 1) * 32, :] = (
                eTb[:, None] * live[None, :] * E[gi * 32:(gi + 1) * 32,
                                                 s_len - 1, :])
        y0[P] = 1.0
        y0[P + 1] = mark[0, s_len - 1, :]
        y0[P + 2] = mark[1, s_len - 1, :]
        in_maps.append({
            "efF": EF.astype(BF),
            "efB": EB.astype(BF),
            "y0": y0.astype(BF),
            "qinitF": qinitF.astype(BF),
            "wblkF": wF.astype(BF),
            "wblkB": wB.astype(BF),
            "onesblkF": obF.astype(BF),
            "onesblkB": obB.astype(BF),
            "onesbc": onesbc.astype(BF),
        })
    return in_maps, Ccum


def _reconstruct(results, Ccum, transition, lengths, s_len=S):
    m = s_len // 2
    n_cores = len(results)
    n_ev = (m - EV0 - 1) // NK + 1 if m > EV0 else 0
    i_apps = EV0 + NK * np.arange(n_ev) + LAG                # (E,)

    out = np.zeros(n_cores * BC, np.float64)
    for core in range(n_cores):
        res = results[core]
        qF = res["qF"].astype(np.float64).reshape(NGRP, 32, FD)
        qB = res["qB"].astype(np.float64).reshape(NGRP, 32, FD)
        lcF = -np.log(np.maximum(
            res["rdF"].astype(np.float64).reshape(NGRP, n_ev, FD), 1e-300))
        lcB = -np.log(np.maximum(
            res["rdB"].astype(np.float64).reshape(NGRP, n_ev, FD), 1e-300))
        for gi in range(NGRP):
            bs = core * BC + gi * FD + np.arange(FD)
            L = lengths[bs]
            dot = (qF[gi] * qB[gi]).sum(axis=0)              # (FD,)
            base = np.log(np.maximum(dot, 1e-300))
            acc = Ccum[L, bs]
            acc = acc + lcF[gi].sum(axis=0)                  # all F events
            i_inj = (s_len - 1) - L                          # -1 when L==s_len
            incB = (i_apps[:, None] >= i_inj[None, :])       # (E, FD)
            acc = acc + (lcB[gi] * incB).sum(axis=0)
            out[bs] = base + acc
    return out


_CACHED_NC = None
LAST_RESULTS = None         # BassKernelResults of the most recent run


def kernel(feats, mask, transition):
    global _CACHED_NC, LAST_RESULTS
    feats = np.asarray(feats, np.float32)
    mask = np.asarray(mask, np.float32)
    transition = np.asarray(transition, np.float32)
    lengths = mask.sum(axis=0).astype(np.int64)              # (B,)

    in_maps, Ccum = _host_prep(feats, transition, lengths)
    if _CACHED_NC is None:
        _CACHED_NC = build_program()
    trace = bool(int(os.environ.get("CRF_TRACE", "0")))
    if trace:
        try:  # supply the NTFF hook module this image's antenv lacks
            import types
            from trn_agent_boot.trn_boot import _ntff_profile_via_ctypes
            if "antenv.axon_hooks" not in sys.modules:
                mm_ = types.ModuleType("antenv.axon_hooks")
                mm_._HOOK = None
                mm_.set_axon_ntff_profile_hook = lambda h: setattr(mm_, "_HOOK", h)
                mm_.get_axon_ntff_profile_hook = lambda: mm_._HOOK
                sys.modules["antenv.axon_hooks"] = mm_
            sys.modules["antenv.axon_hooks"].set_axon_ntff_profile_hook(
                _ntff_profile_via_ctypes("/opt/axon/libaxon_pjrt.so"))
        except Exception as e:  # profiling degrades, run still works
            print(f"ntff hook registration failed: {e}")
    res = run_bass_kernel_spmd(_CACHED_NC, in_maps, core_ids=list(range(NCORES)),
                               trace=trace)
    LAST_RESULTS = res
    out = _reconstruct(res.results, Ccum, transition, lengths)
    return out.astype(np.float32)


if __name__ == "__main__":
    feats = np.load("/tmp/in_feats.npy")
    mask = np.load("/tmp/in_mask.npy")
    trans = np.load("/tmp/in_transition.npy")
    got = kernel(feats, mask, trans)
    exp = np.load("/tmp/expected.npy")
    rel = np.abs(got - exp) / np.maximum(1.0, np.abs(exp))
    print("max rel:", rel.max(), "mean:", rel.mean())
